# revision 10
# baseline (speedup 1.0000x reference)
"""Trainium2 Bass kernel for nn_BiSTGNNv7 (8-core SPMD).

Sharding:
  Phase A (batch-parallel, core b owns batch b): spatial encoder, temporal
  GRU (hidden 32), 2x TN layers (dynamic adjacency tanh(relu(X X^T)) + 2
  GCN hops), feature rebuild. Feature-on-partition (transposed) layouts
  throughout, so no on-chip transposes are needed.
  Phase B (hidden-parallel): the time-rebuild GRU (hidden 2000, padded to
  2048) is sharded 8 ways over the hidden dim; each step computes this
  core's 256 hidden units for all 8 batches, then an AllGather
  redistributes h for the next step. Gate math runs on combined
  (128, 16) tiles (both 128-row halves of the slice as column groups), and
  the h exchange uses a single fat (128,16) staging DMA each way. The
  out_W-weighted sum of hidden states is accumulated on the fly, returned
  per-slice, and combined with the feature-rebuild output on the host.

All matmul operands are fp16 (fp32 PSUM accumulation).
"""
import sys
sys.path.insert(0, '/opt/trn_rl_repo')

import numpy as np
import concourse.bacc as bacc
import concourse.mybir as mybir
import concourse.tile as tile
from concourse.bass_utils import run_bass_kernel_spmd
from concourse.alu_op_type import AluOpType
from concourse.mybir import ActivationFunctionType as AF

F16 = mybir.dt.float16
F32 = mybir.dt.float32

NC = 8          # cores
B, T, N = 8, 96, 2000
L, E, Dt = 32, 32, 4
NG = N + T      # 2096 graph nodes
NGP = 2176      # padded graph nodes (17*128)
NB = NGP // 128  # 17 node blocks
NH = 2048       # padded GRU hidden
SL = NH // NC   # 256 hidden units per core
GR = 3 * SL     # 768 gate rows per core
KC = NH // 128  # 16 contraction chunks

_CACHE = {}


def _chunks(total, step):
    out, s = [], 0
    while s < total:
        out.append((s, min(step, total - s)))
        s += step
    return out


def build_program():
    nc = bacc.Bacc("TRN2", target_bir_lowering=False, debug=False,
                   num_devices=NC)
    dram = {}

    def din(name, shape, dt):
        dram[name] = nc.dram_tensor(name, list(shape), dt,
                                    kind="ExternalInput")
        return dram[name]

    din("xT", (T, N), F16)
    din("spembT", (E, N), F16)
    din("xtT", (NH, T), F16)
    din("spW1", (T + E, L), F16)
    din("spb1", (L, 1), F32)
    din("spW2", (L, L), F16)
    din("spb2", (L, 1), F32)
    din("tWihT", (NH, 3 * L), F16)
    din("tWhhT", (L, 3 * L), F16)
    din("tgib", (L, 3), F32)
    din("tbhhn", (L, 1), F32)
    din("gcnW", (L, 4 * L), F16)
    din("gcnB", (L, 4), F32)
    din("frW1", (L, T), F16)
    din("frb1", (T, 1), F32)
    din("fvv", (T, 1), F16)
    din("wihTs", (L, GR), F16)
    din("whhTs", (NH, GR), F16)
    din("gibs", (GR, 1), F32)
    din("bhhn16", (128, 16), F32)
    din("outWb", (128, T), F32)
    out_d = nc.dram_tensor("out_d", [NH, 1], F32, kind="ExternalOutput")
    acc_d = nc.dram_tensor("acc_d", [128, 16], F32, kind="ExternalOutput")

    def elu_from_psum(nc, pool, ps_ap, bias_ap, out_ap, tag):
        # elu(x+b) = relu(x+b) + exp(min(x+b, 0)) - 1
        p, f = ps_ap.shape
        t_min = pool.tile([p, f], F32, tag=f"{tag}_min", name=f"{tag}_min")
        t_exp = pool.tile([p, f], F32, tag=f"{tag}_exp", name=f"{tag}_exp")
        t_rel = pool.tile([p, f], F32, tag=f"{tag}_rel", name=f"{tag}_rel")
        nc.vector.tensor_scalar(t_min[:], ps_ap, bias_ap, 0.0,
                                AluOpType.add, AluOpType.min)
        nc.scalar.activation(t_exp[:], t_min[:], AF.Exp)
        nc.vector.tensor_scalar(t_rel[:], ps_ap, bias_ap, 0.0,
                                AluOpType.add, AluOpType.max)
        nc.vector.scalar_tensor_tensor(out_ap, t_exp[:], -1.0, t_rel[:],
                                       AluOpType.add, AluOpType.add)

    with tile.TileContext(nc) as tc:
        with (
            tc.tile_pool(name="cst", bufs=1) as cst,
            tc.tile_pool(name="pa", bufs=1) as pa,
            tc.tile_pool(name="tmp", bufs=3) as tmp,
            tc.tile_pool(name="pb", bufs=2) as pb,
            tc.tile_pool(name="dpool", bufs=1, space="DRAM") as dpool,
        ):
            # ---- constants ----
            spW1_sb = cst.tile([128, L], F16)
            nc.sync.dma_start(spW1_sb[:], dram["spW1"][:])
            spW2_sb = cst.tile([L, L], F16)
            nc.sync.dma_start(spW2_sb[:], dram["spW2"][:])
            spb1_sb = cst.tile([L, 1], F32)
            nc.sync.dma_start(spb1_sb[:], dram["spb1"][:])
            spb2_sb = cst.tile([L, 1], F32)
            nc.sync.dma_start(spb2_sb[:], dram["spb2"][:])
            tWihT_sb = cst.tile([128, KC * 3 * L], F16)
            nc.sync.dma_start(
                tWihT_sb[:].rearrange("p (k g) -> p k g", k=KC),
                dram["tWihT"][:].rearrange("(k p) g -> p k g", p=128))
            tWhhT_sb = cst.tile([L, 3 * L], F16)
            nc.sync.dma_start(tWhhT_sb[:], dram["tWhhT"][:])
            tgib_sb = cst.tile([L, 3], F32)
            nc.sync.dma_start(tgib_sb[:], dram["tgib"][:])
            tbhhn_sb = cst.tile([L, 1], F32)
            nc.sync.dma_start(tbhhn_sb[:], dram["tbhhn"][:])
            gcnW_sb = cst.tile([L, 4 * L], F16)
            nc.sync.dma_start(gcnW_sb[:], dram["gcnW"][:])
            gcnB_sb = cst.tile([L, 4], F32)
            nc.sync.dma_start(gcnB_sb[:], dram["gcnB"][:])
            frW1_sb = cst.tile([L, T], F16)
            nc.sync.dma_start(frW1_sb[:], dram["frW1"][:])
            frb1_sb = cst.tile([T, 1], F32)
            nc.sync.dma_start(frb1_sb[:], dram["frb1"][:])
            fvv_sb = cst.tile([T, 1], F16)
            nc.sync.dma_start(fvv_sb[:], dram["fvv"][:])
            wihTs_sb = cst.tile([L, GR], F16)
            nc.sync.dma_start(wihTs_sb[:], dram["wihTs"][:])
            wt_sb = cst.tile([128, 6 * KC * 128], F16)
            nc.sync.dma_start(
                wt_sb[:].rearrange("p (k m) -> p k m", k=KC),
                dram["whhTs"][:].rearrange("(k p) m -> p k m", p=128))
            gibs_sb = cst.tile([128, 6], F32)
            nc.sync.dma_start(
                gibs_sb[:].rearrange("p (m one) -> p m one", m=6),
                dram["gibs"][:].rearrange("(m p) one -> p m one", p=128))
            bhhn16_sb = cst.tile([128, 16], F32)
            nc.sync.dma_start(bhhn16_sb[:], dram["bhhn16"][:])
            outWb_sb = cst.tile([128, T], F32)
            nc.sync.dma_start(outWb_sb[:], dram["outWb"][:])

            # ---- phase A inputs ----
            xsT_sb = pa.tile([128, N], F16)
            nc.sync.dma_start(xsT_sb[0:T, :], dram["xT"][:])
            nc.sync.dma_start(xsT_sb[T:128, :], dram["spembT"][:])
            xtT_sb = pa.tile([128, KC * T], F16)
            nc.sync.dma_start(
                xtT_sb[:].rearrange("p (k t) -> p k t", k=KC),
                dram["xtT"][:].rearrange("(k p) t -> p k t", p=128))

            # X^T split: node order [Xt(96)+pad(32) | Xs(2000) | pad(48)]
            XT0t = pa.tile([L, 128], F16)
            XT1t = pa.tile([L, 128], F16)
            XT0s = pa.tile([L, 2048], F16)
            XT1s = pa.tile([L, 2048], F16)
            nc.vector.memset(XT0t[:, T:128], 0.0)
            nc.vector.memset(XT0s[:, N:2048], 0.0)
            Xs1 = pa.tile([L, N], F16)
            A_sb = [pa.tile([128, NGP], F16, name=f"A{i}")
                    for i in range(NB)]
            V_sb = pa.tile([128, NB * L], F16)
            HT = pa.tile([T, NH], F16)
            nc.vector.memset(HT[:, N:NH], 0.0)
            o_sb = pa.tile([128, 16], F32)
            gi_sb = pa.tile([L, 3 * T], F32)

            with tc.tile_pool(name="psA", bufs=1, space="PSUM") as psA:
                # spatial encoder
                for (c0, cw) in _chunks(N, 512):
                    ps1 = psA.tile([L, 512], F32, tag="ps1", name="ps1", bufs=2)
                    nc.tensor.matmul(ps1[:, 0:cw], spW1_sb[:],
                                     xsT_sb[:, c0:c0 + cw],
                                     start=True, stop=True)
                    elu_from_psum(nc, tmp, ps1[:, 0:cw], spb1_sb[:, 0:1],
                                  Xs1[:, c0:c0 + cw], "se")
                for (c0, cw) in _chunks(N, 512):
                    ps2 = psA.tile([L, 512], F32, tag="ps2", name="ps2", bufs=2)
                    nc.tensor.matmul(ps2[:, 0:cw], spW2_sb[:],
                                     Xs1[:, c0:c0 + cw],
                                     start=True, stop=True)
                    nc.scalar.activation(XT0s[:, c0:c0 + cw], ps2[:, 0:cw],
                                         AF.Identity, bias=spb2_sb[:, 0:1])

                # temporal encoder: gi for all steps (per-gate, base 0)
                for gg in range(3):
                    psgi = psA.tile([L, T], F32, tag="psgi",
                                    name=f"psgi{gg}")
                    for k in range(KC):
                        nc.tensor.matmul(
                            psgi[:],
                            tWihT_sb[:, k * 3 * L + L * gg:
                                     k * 3 * L + L * (gg + 1)],
                            xtT_sb[:, k * T:(k + 1) * T],
                            start=(k == 0), stop=(k == KC - 1))
                    nc.scalar.activation(gi_sb[:, T * gg:T * (gg + 1)],
                                         psgi[:], AF.Identity,
                                         bias=tgib_sb[:, gg:gg + 1])

                # temporal recurrence (hidden 32), fp16 state in XT0t cols
                h0_16 = tmp.tile([L, 1], F16, tag="h016", name="h016")
                nc.vector.memset(h0_16[:], 0.0)
                hcol = h0_16[:]
                for t in range(T):
                    psg = [psA.tile([L, 1], F32, tag=f"psg{gg}",
                                    bufs=1, name=f"psg{gg}_{t}")
                           for gg in range(3)]
                    for gg in range(3):
                        nc.tensor.matmul(psg[gg][:],
                                         tWhhT_sb[:, L * gg:L * (gg + 1)],
                                         hcol, start=True, stop=True)
                    r = tmp.tile([L, 1], F32, tag="t_r", name="t_r")
                    nc.scalar.activation(r[:], psg[0][:], AF.Sigmoid,
                                         bias=gi_sb[:, t:t + 1])
                    z = tmp.tile([L, 1], F32, tag="t_z", name="t_z")
                    nc.scalar.activation(z[:], psg[1][:], AF.Sigmoid,
                                         bias=gi_sb[:, T + t:T + t + 1])
                    t6 = tmp.tile([L, 1], F32, tag="t_t6", name="t_t6")
                    nc.vector.scalar_tensor_tensor(
                        t6[:], psg[2][:], tbhhn_sb[:, 0:1], r[:],
                        AluOpType.add, AluOpType.mult)
                    nt = tmp.tile([L, 1], F32, tag="t_nt", name="t_nt")
                    nc.scalar.activation(nt[:], t6[:], AF.Tanh,
                                         bias=gi_sb[:, 2 * T + t:2 * T + t + 1])
                    d = tmp.tile([L, 1], F32, tag="t_d", name="t_d")
                    nc.vector.tensor_tensor(d[:], nt[:], hcol,
                                            AluOpType.subtract)
                    zd = tmp.tile([L, 1], F32, tag="t_zd", name="t_zd")
                    nc.vector.tensor_tensor(zd[:], z[:], d[:],
                                            AluOpType.mult)
                    # h_new = nt - z*(nt - h) = (1-z)*nt + z*h
                    nc.vector.tensor_tensor(XT0t[:, t:t + 1], nt[:], zd[:],
                                            AluOpType.subtract)
                    hcol = XT0t[:, t:t + 1]

            # ---- TN layers ----
            # lhsT/rhs/output segments: seg 0 = temporal tile (128 cols),
            # segs 1+ = spatial tile (4x512). Spatial-only matmuls are
            # emitted first so they can overlap the temporal recurrence.
            XTat, XTas, XTbt, XTbs = XT0t, XT0s, XT1t, XT1s

            def _lhs(i, xtt, xts):
                return xtt[:, 0:128] if i == 0 else                     xts[:, 128 * (i - 1):128 * i]

            def _segs(xtt, xts):
                segs = [(xtt, 0, 0, 128)]
                for (c0, cw) in _chunks(2048, 512):
                    segs.append((xts, c0, 128 + c0, cw))
                return segs

            for ti in range(2):
                with tc.tile_pool(name=f"psS{ti}", bufs=8,
                                  space="PSUM") as psS:
                    segs = _segs(XTat, XTas)
                    order = [(i, s) for i in range(1, NB)
                             for s in range(1, 5)]
                    order += [(0, s) for s in range(5)]
                    order += [(i, 0) for i in range(1, NB)]
                    for (i, s) in order:
                        (xt, c0, a0, cw) = segs[s]
                        ps = psS.tile([128, 512], F32, tag="psS",
                                      name="psS")
                        nc.tensor.matmul(ps[:, 0:cw],
                                         _lhs(i, XTat, XTas),
                                         xt[:, c0:c0 + cw],
                                         start=True, stop=True)
                        th = tmp.tile([128, 512], F16, tag="th",
                                      name="th")
                        nc.scalar.activation(th[:, 0:cw], ps[:, 0:cw],
                                             AF.Tanh)
                        nc.vector.tensor_scalar_max(A_sb[i][:, a0:a0 + cw],
                                                    th[:, 0:cw], 0.0)
                for li in range(2):
                    w0 = 32 * (2 * ti + li)
                    with tc.tile_pool(name=f"psG{ti}{li}", bufs=4,
                                      space="PSUM") as psG:
                        for j in list(range(1, NB)) + [0]:
                            psv = psG.tile([128, L], F32, tag="psv",
                                           name="psv")
                            nc.tensor.matmul(psv[:],
                                             _lhs(j, XTat, XTas),
                                             gcnW_sb[:, w0:w0 + L],
                                             start=True, stop=True)
                            nc.vector.tensor_copy(
                                V_sb[:, L * j:L * (j + 1)], psv[:])
                        for (xt, c0, a0, cw) in _segs(XTbt, XTbs):
                            psa = psG.tile([L, 512], F32, tag="psa",
                                           name="psa")
                            for j in range(NB):
                                nc.tensor.matmul(psa[:, 0:cw],
                                                 V_sb[:, L * j:L * (j + 1)],
                                                 A_sb[j][:, a0:a0 + cw],
                                                 start=(j == 0),
                                                 stop=(j == NB - 1))
                            elu_from_psum(nc, tmp, psa[:, 0:cw],
                                          gcnB_sb[:, 2 * ti + li:2 * ti + li + 1],
                                          xt[:, c0:c0 + cw], "ge")
                    nc.vector.memset(XTbt[:, T:128], 0.0)
                    nc.vector.memset(XTbs[:, N:2048], 0.0)
                    XTat, XTbt = XTbt, XTat
                    XTas, XTbs = XTbs, XTas

            # ---- feature rebuild + o_s ----
            with tc.tile_pool(name="psF", bufs=4, space="PSUM") as psF:
                for (c0, cw) in _chunks(N, 512):
                    psh = psF.tile([T, 512], F32, tag="psh", name="psh")
                    nc.tensor.matmul(psh[:, 0:cw], frW1_sb[:],
                                     XTas[:, c0:c0 + cw],
                                     start=True, stop=True)
                    elu_from_psum(nc, tmp, psh[:, 0:cw], frb1_sb[:, 0:1],
                                  HT[:, c0:c0 + cw], "fr")
                for j in range(16):
                    pso = psF.tile([128, 1], F32, tag="pso", name="pso")
                    nc.tensor.matmul(pso[:], HT[:, 128 * j:128 * (j + 1)],
                                     fvv_sb[:], start=True, stop=True)
                    nc.vector.tensor_copy(o_sb[:, j:j + 1], pso[:])
            nc.sync.dma_start(
                out_d[:].rearrange("(j p) one -> p j one", p=128),
                o_sb[:].rearrange("p (j one) -> p j one", j=16))

            # ---- AllGather Xt2, then gi for the big GRU ----
            xt2_d = dpool.tile([L, T], F16)
            nc.sync.dma_start(xt2_d[:], XTat[:, 0:T])
            xt2a_d = dpool.tile([NC * L, T], F16)
            nc.gpsimd.collective_compute(
                "AllGather", AluOpType.bypass,
                replica_groups=[list(range(NC))],
                ins=[xt2_d[:].opt()], outs=[xt2a_d[:].opt()])
            xt2a_sb = pa.tile([L, NC * T], F16)
            nc.sync.dma_start(
                xt2a_sb[:].rearrange("f (b t) -> f b t", b=NC),
                xt2a_d[:].rearrange("(b f) t -> f b t", f=L))

            # gi for the big GRU: per gate g, (128, 16*T) tiles with
            # column layout [t][half h][batch b] -> col 16t + 8h + b
            gi_g = [pa.tile([128, 16 * T], F32, name=f"gig{g}")
                    for g in range(3)]
            with tc.tile_pool(name="psI", bufs=6, space="PSUM") as psI:
                for b in range(NC):
                    for g in range(3):
                        for h in range(2):
                            m = 2 * g + h
                            psi = psI.tile([128, T], F32, tag="psi",
                                           name="psi")
                            nc.tensor.matmul(
                                psi[:], wihTs_sb[:, 128 * m:128 * (m + 1)],
                                xt2a_sb[:, T * b:T * (b + 1)],
                                start=True, stop=True)
                            nc.scalar.activation(
                                gi_g[g][:, 8 * h + b:16 * T:16], psi[:],
                                AF.Identity, bias=gibs_sb[:, m:m + 1])

            # ---- phase B: hidden-sharded big GRU ----
            # h slice state: (128, 16) f16 = [half h][batch b] columns
            hprev = pb.tile([128, 16], F16, tag="h16", name="h16_0")
            nc.vector.memset(hprev[:], 0.0)
            accv = pb.tile([128, 16], F32, tag="acc", name="acc_0")
            nc.vector.memset(accv[:], 0.0)
            hall_lo = pb.tile([128, 64], F16, tag="hlo", name="hlo_i")
            hall_hi = pb.tile([128, 64], F16, tag="hhi", name="hhi_i")
            nc.vector.memset(hall_lo[:], 0.0)
            nc.vector.memset(hall_hi[:], 0.0)

            with tc.tile_pool(name="psB", bufs=6, space="PSUM") as psB:
                for t in range(T):
                    pst = [psB.tile([128, 16], F32, tag=f"psB{g}",
                                    name=f"ps_{t}_{g}", bufs=2)
                           for g in range(3)]
                    # all lo-half contraction chunks first (across every
                    # gate group), so the hi-half hall DMA overlaps them
                    for klo in (True, False):
                        for g in (2, 0, 1):
                            for h in range(2):
                                m = 2 * g + h
                                for k in (range(8) if klo else range(8, KC)):
                                    j = m * KC + k
                                    rhs = (hall_lo[:, 8 * k:8 * k + 8]
                                           if k < 8 else
                                           hall_hi[:, 8 * (k - 8):
                                                    8 * (k - 7)])
                                    nc.tensor.matmul(
                                        pst[g][:, 8 * h:8 * h + 8],
                                        wt_sb[:, 128 * j:128 * (j + 1)],
                                        rhs,
                                        start=(k == 0),
                                        stop=(k == KC - 1))
                    gi_t = [gi_g[g][:, 16 * t:16 * t + 16] for g in range(3)]
                    bn = tmp.tile([128, 16], F32, tag="b_bn", name="b_bn")
                    nc.vector.tensor_tensor(bn[:], pst[2][:], bhhn16_sb[:],
                                            AluOpType.add)
                    rp = tmp.tile([128, 16], F32, tag="b_rp", name="b_rp")
                    nc.vector.tensor_tensor(rp[:], pst[0][:], gi_t[0],
                                            AluOpType.add)
                    r = tmp.tile([128, 16], F32, tag="b_r", name="b_r")
                    nc.scalar.activation(r[:], rp[:], AF.Sigmoid)
                    zp = tmp.tile([128, 16], F32, tag="b_zp", name="b_zp")
                    nc.vector.tensor_tensor(zp[:], pst[1][:], gi_t[1],
                                            AluOpType.add)
                    z = tmp.tile([128, 16], F32, tag="b_z", name="b_z")
                    nc.scalar.activation(z[:], zp[:], AF.Sigmoid)
                    t6 = tmp.tile([128, 16], F32, tag="b_t6", name="b_t6")
                    nc.vector.tensor_tensor(t6[:], bn[:], r[:],
                                            AluOpType.mult)
                    npre = tmp.tile([128, 16], F32, tag="b_np", name="b_np")
                    nc.vector.tensor_tensor(npre[:], t6[:], gi_t[2],
                                            AluOpType.add)
                    nt = tmp.tile([128, 16], F32, tag="b_nt", name="b_nt")
                    nc.scalar.activation(nt[:], npre[:], AF.Tanh)
                    d = tmp.tile([128, 16], F32, tag="b_d", name="b_d")
                    nc.vector.tensor_tensor(d[:], hprev[:], nt[:],
                                            AluOpType.subtract)
                    zd = tmp.tile([128, 16], F32, tag="b_zd", name="b_zd")
                    nc.vector.tensor_tensor(zd[:], z[:], d[:],
                                            AluOpType.mult)
                    hnew = pb.tile([128, 16], F16, tag="h16",
                                   name=f"hnew_{t}")
                    nc.vector.tensor_tensor(hnew[:], nt[:], zd[:],
                                            AluOpType.add)
                    if t < T - 1:
                        agin = dpool.tile([128, 16], F16, tag=f"agi{t}",
                                          name=f"agi{t}")
                        nc.sync.dma_start(agin[:], hnew[:])
                    anew = pb.tile([128, 16], F32, tag="acc",
                                   name=f"anew_{t}")
                    nc.vector.scalar_tensor_tensor(
                        anew[:], hnew[:], outWb_sb[:, t:t + 1], accv[:],
                        AluOpType.mult, AluOpType.add)
                    accv = anew
                    hprev = hnew
                    if t == T - 1:
                        break
                    agout = dpool.tile([NC * 128, 16], F16, tag=f"ago{t}",
                                       name=f"ago{t}")
                    nc.gpsimd.collective_compute(
                        "AllGather", AluOpType.bypass,
                        replica_groups=[list(range(NC))],
                        ins=[agin[:].opt()], outs=[agout[:].opt()],
                        unique_tensors="Yes")
                    hall_lo = pb.tile([128, 64], F16, tag="hlo",
                                      name=f"hlo{t}")
                    hall_hi = pb.tile([128, 64], F16, tag="hhi",
                                      name=f"hhi{t}")
                    nc.sync.dma_start(
                        hall_lo[:].rearrange("p (c m) -> p c m", c=4),
                        agout[0:512, :].rearrange("(c p) m -> p c m", p=128))
                    nc.scalar.dma_start(
                        hall_hi[:].rearrange("p (c m) -> p c m", c=4),
                        agout[512:1024, :].rearrange("(c p) m -> p c m",
                                                     p=128))

            # ---- write acc slice; host does the final combine ----
            nc.sync.dma_start(acc_d[:], accv[:])

    nc.compile()
    return nc


def _prep_inputs(inputs):
    f16 = np.float16
    f32 = np.float32
    x = np.asarray(inputs['x'], f32)
    x_mark = np.asarray(inputs['x_mark'], f32)
    g = {k: np.asarray(v, f32) for k, v in inputs.items()}

    # temporal-GRU input weights, transposed + padded to NH rows
    tWihT = np.zeros((NH, 3 * L), f16)
    tWihT[:N + Dt + E, :] = g['t_Wih'].T.astype(f16)
    rz_mask = np.concatenate([np.ones(2 * L, f32), np.zeros(L, f32)])
    tgib = (g['t_bih'] + g['t_bhh'] * rz_mask).reshape(3, L).T.astype(f32)

    # big GRU: pad hidden 2000->2048, group gates [r;z;n] per slice
    Whh_p = np.zeros((3 * NH, NH), f32)
    Wih_p = np.zeros((3 * NH, L), f32)
    bih_p = np.zeros(3 * NH, f32)
    bhh_p = np.zeros(3 * NH, f32)
    for gi in range(3):
        Whh_p[gi * NH:gi * NH + N, :N] = g['tr_Whh'][gi * N:(gi + 1) * N, :]
        Wih_p[gi * NH:gi * NH + N, :] = g['tr_Wih'][gi * N:(gi + 1) * N, :]
        bih_p[gi * NH:gi * NH + N] = g['tr_bih'][gi * N:(gi + 1) * N]
        bhh_p[gi * NH:gi * NH + N] = g['tr_bhh'][gi * N:(gi + 1) * N]

    fvv = (g['fr_W2'] @ g['out_W']).astype(f16)
    fb = float((g['fr_b2'] @ g['out_W'])[0] + g['out_b'][0])
    outWb = np.tile(g['out_W'][:, 0][None, :], (128, 1)).astype(f32)

    common = {
        'spembT': g['sp_emb'].T.astype(f16),
        'spW1': g['sp_W1'].astype(f16),
        'spb1': g['sp_b1'][:, None].astype(f32),
        'spW2': g['sp_W2'].astype(f16),
        'spb2': g['sp_b2'][:, None].astype(f32),
        'tWihT': tWihT,
        'tWhhT': g['t_Whh'].T.astype(f16),
        'tgib': tgib,
        'tbhhn': g['t_bhh'][2 * L:3 * L][:, None].astype(f32),
        'gcnW': np.concatenate([g['gcn_W'][ti, li] for ti in range(2) for li in range(2)], axis=1).astype(f16),
        'gcnB': np.stack([g['gcn_b'][ti, li] for ti in range(2) for li in range(2)], axis=1).astype(f32),
        'frW1': g['fr_W1'].astype(f16),
        'frb1': g['fr_b1'][:, None].astype(f32),
        'fvv': fvv,
        'outWb': outWb,
    }

    in_maps = []
    for c in range(NC):
        rows = np.concatenate([np.arange(c * SL, (c + 1) * SL) + gi * NH
                               for gi in range(3)])
        xtT = np.zeros((NH, T), f16)
        xtT[:N, :] = x[c].T.astype(f16)
        xtT[N:N + Dt, :] = x_mark[c].T.astype(f16)
        xtT[N + Dt:N + Dt + E, :] = g['t_emb'].T.astype(f16)
        m = dict(common)
        m['xT'] = x[c].astype(f16)
        m['xtT'] = xtT
        m['wihTs'] = Wih_p[rows, :].T.astype(f16).copy()
        m['whhTs'] = Whh_p[rows, :].T.astype(f16).copy()
        gib = bih_p[rows] + bhh_p[rows] * np.concatenate(
            [np.ones(2 * SL, f32), np.zeros(SL, f32)])
        m['gibs'] = gib[:, None].astype(f32)
        # bhh_n for this slice, (128 rows, [half h][batch b]) columns
        bn = bhh_p[rows[2 * SL:]]  # (256,)
        m['bhhn16'] = np.concatenate(
            [np.tile(bn[128 * h:128 * (h + 1)][:, None], (1, 8))
             for h in range(2)], axis=1).astype(f32)
        in_maps.append(m)
    return in_maps, fb


def kernel(**inputs):
    if 'nc' not in _CACHE:
        _CACHE['nc'] = build_program()
    nc = _CACHE['nc']
    in_maps, fb = _prep_inputs(inputs)
    res = run_bass_kernel_spmd(nc, in_maps, core_ids=list(range(NC)))
    _CACHE['last_res'] = res
    o_s = np.stack([res.results[b]['out_d'][:N, 0] for b in range(NC)])
    # acc_d per core: (128, 16) cols = [half h][batch b]; hidden row
    # n = c*256 + h*128 + p
    acc_full = np.zeros((NH, B), np.float32)
    for c in range(NC):
        a = res.results[c]['acc_d']
        for h in range(2):
            acc_full[c * SL + 128 * h:c * SL + 128 * (h + 1), :] = \
                a[:, 8 * h:8 * h + 8]
    out = o_s + acc_full[:N, :].T + fb
    return out[:, :, None].astype(np.float32)


# revision 12
# speedup vs baseline: 1.0139x; 1.0139x over previous
"""Trainium2 Bass kernel for nn_BiSTGNNv7 (8-core SPMD).

Sharding:
  Phase A (batch-parallel, core b owns batch b): spatial encoder, temporal
  GRU (hidden 32), 2x TN layers (dynamic adjacency tanh(relu(X X^T)) + 2
  GCN hops), feature rebuild. Feature-on-partition (transposed) layouts
  throughout, so no on-chip transposes are needed.
  Phase B (hidden-parallel): the time-rebuild GRU (hidden 2000, padded to
  2048) is sharded 8 ways over the hidden dim; each step computes this
  core's 256 hidden units for all 8 batches, then an AllGather
  redistributes h for the next step. Gate math runs on combined
  (128, 16) tiles (both 128-row halves of the slice as column groups), and
  the h exchange uses a single fat (128,16) staging DMA each way. The
  out_W-weighted sum of hidden states is accumulated on the fly, returned
  per-slice, and combined with the feature-rebuild output on the host.

All matmul operands are fp16 (fp32 PSUM accumulation).
"""
import sys
sys.path.insert(0, '/opt/trn_rl_repo')

import numpy as np
import concourse.bacc as bacc
import concourse.mybir as mybir
import concourse.tile as tile
from concourse.bass_utils import run_bass_kernel_spmd
from concourse.alu_op_type import AluOpType
from concourse.mybir import ActivationFunctionType as AF

F16 = mybir.dt.float16
F32 = mybir.dt.float32

NC = 8          # cores
B, T, N = 8, 96, 2000
L, E, Dt = 32, 32, 4
NG = N + T      # 2096 graph nodes
NGP = 2176      # padded graph nodes (17*128)
NB = NGP // 128  # 17 node blocks
NH = 2048       # padded GRU hidden
SL = NH // NC   # 256 hidden units per core
GR = 3 * SL     # 768 gate rows per core
KC = NH // 128  # 16 contraction chunks

_CACHE = {}


def _chunks(total, step):
    out, s = [], 0
    while s < total:
        out.append((s, min(step, total - s)))
        s += step
    return out


def build_program():
    nc = bacc.Bacc("TRN2", target_bir_lowering=False, debug=False,
                   num_devices=NC)
    dram = {}

    def din(name, shape, dt):
        dram[name] = nc.dram_tensor(name, list(shape), dt,
                                    kind="ExternalInput")
        return dram[name]

    din("xT", (T, N), F16)
    din("spembT", (E, N), F16)
    din("xtT", (NH, T), F16)
    din("spW1", (T + E, L), F16)
    din("spb1", (L, 1), F32)
    din("spW2", (L, L), F16)
    din("spb2", (L, 1), F32)
    din("tWihT", (NH, 3 * L), F16)
    din("tWhhT", (L, 3 * L), F16)
    din("tgib", (L, 3), F32)
    din("tbhhn", (L, 1), F32)
    din("gcnW", (L, 4 * L), F16)
    din("gcnB", (L, 4), F32)
    din("frW1", (L, T), F16)
    din("frb1", (T, 1), F32)
    din("fvv", (T, 1), F16)
    din("wihTs", (L, GR), F16)
    din("whhTs", (NH, GR), F16)
    din("gibs", (GR, 1), F32)
    din("bhhn16", (128, 16), F32)
    din("outWb", (128, T), F32)
    out_d = nc.dram_tensor("out_d", [NH, 1], F32, kind="ExternalOutput")
    acc_d = nc.dram_tensor("acc_d", [128, 16], F32, kind="ExternalOutput")

    def elu_from_psum(nc, pool, ps_ap, bias_ap, out_ap, tag):
        # elu(x+b) = relu(x+b) + exp(min(x+b, 0)) - 1
        p, f = ps_ap.shape
        t_min = pool.tile([p, f], F32, tag=f"{tag}_min", name=f"{tag}_min")
        t_exp = pool.tile([p, f], F32, tag=f"{tag}_exp", name=f"{tag}_exp")
        t_rel = pool.tile([p, f], F32, tag=f"{tag}_rel", name=f"{tag}_rel")
        nc.vector.tensor_scalar(t_min[:], ps_ap, bias_ap, 0.0,
                                AluOpType.add, AluOpType.min)
        nc.scalar.activation(t_exp[:], t_min[:], AF.Exp)
        nc.vector.tensor_scalar(t_rel[:], ps_ap, bias_ap, 0.0,
                                AluOpType.add, AluOpType.max)
        nc.vector.scalar_tensor_tensor(out_ap, t_exp[:], -1.0, t_rel[:],
                                       AluOpType.add, AluOpType.add)

    with tile.TileContext(nc) as tc:
        with (
            tc.tile_pool(name="cst", bufs=1) as cst,
            tc.tile_pool(name="pa", bufs=1) as pa,
            tc.tile_pool(name="tmp", bufs=3) as tmp,
            tc.tile_pool(name="pb", bufs=2) as pb,
            tc.tile_pool(name="dpool", bufs=1, space="DRAM") as dpool,
        ):
            # ---- constants ----
            spW1_sb = cst.tile([128, L], F16)
            nc.sync.dma_start(spW1_sb[:], dram["spW1"][:])
            spW2_sb = cst.tile([L, L], F16)
            nc.sync.dma_start(spW2_sb[:], dram["spW2"][:])
            spb1_sb = cst.tile([L, 1], F32)
            nc.sync.dma_start(spb1_sb[:], dram["spb1"][:])
            spb2_sb = cst.tile([L, 1], F32)
            nc.sync.dma_start(spb2_sb[:], dram["spb2"][:])
            tWihT_sb = cst.tile([128, KC * 3 * L], F16)
            nc.sync.dma_start(
                tWihT_sb[:].rearrange("p (k g) -> p k g", k=KC),
                dram["tWihT"][:].rearrange("(k p) g -> p k g", p=128))
            tWhhT_sb = cst.tile([L, 3 * L], F16)
            nc.sync.dma_start(tWhhT_sb[:], dram["tWhhT"][:])
            tgib_sb = cst.tile([L, 3], F32)
            nc.sync.dma_start(tgib_sb[:], dram["tgib"][:])
            tbhhn_sb = cst.tile([L, 1], F32)
            nc.sync.dma_start(tbhhn_sb[:], dram["tbhhn"][:])
            gcnW_sb = cst.tile([L, 4 * L], F16)
            nc.sync.dma_start(gcnW_sb[:], dram["gcnW"][:])
            gcnB_sb = cst.tile([L, 4], F32)
            nc.sync.dma_start(gcnB_sb[:], dram["gcnB"][:])
            frW1_sb = cst.tile([L, T], F16)
            nc.sync.dma_start(frW1_sb[:], dram["frW1"][:])
            frb1_sb = cst.tile([T, 1], F32)
            nc.sync.dma_start(frb1_sb[:], dram["frb1"][:])
            fvv_sb = cst.tile([T, 1], F16)
            nc.sync.dma_start(fvv_sb[:], dram["fvv"][:])
            wihTs_sb = cst.tile([L, GR], F16)
            nc.sync.dma_start(wihTs_sb[:], dram["wihTs"][:])
            wt_sb = cst.tile([128, 6 * KC * 128], F16)
            nc.sync.dma_start(
                wt_sb[:].rearrange("p (k m) -> p k m", k=KC),
                dram["whhTs"][:].rearrange("(k p) m -> p k m", p=128))
            gibs_sb = cst.tile([128, 6], F32)
            nc.sync.dma_start(
                gibs_sb[:].rearrange("p (m one) -> p m one", m=6),
                dram["gibs"][:].rearrange("(m p) one -> p m one", p=128))
            bhhn16_sb = cst.tile([128, 16], F32)
            nc.sync.dma_start(bhhn16_sb[:], dram["bhhn16"][:])
            outWb_sb = cst.tile([128, T], F32)
            nc.sync.dma_start(outWb_sb[:], dram["outWb"][:])

            # ---- phase A inputs ----
            xsT_sb = pa.tile([128, N], F16)
            nc.sync.dma_start(xsT_sb[0:T, :], dram["xT"][:])
            nc.sync.dma_start(xsT_sb[T:128, :], dram["spembT"][:])
            xtT_sb = pa.tile([128, KC * T], F16)
            nc.sync.dma_start(
                xtT_sb[:].rearrange("p (k t) -> p k t", k=KC),
                dram["xtT"][:].rearrange("(k p) t -> p k t", p=128))

            # X^T split: node order [Xt(96)+pad(32) | Xs(2000) | pad(48)]
            XT0t = pa.tile([L, 128], F16)
            XT1t = pa.tile([L, 128], F16)
            XT0s = pa.tile([L, 2048], F16)
            XT1s = pa.tile([L, 2048], F16)
            nc.vector.memset(XT0t[:, T:128], 0.0)
            nc.vector.memset(XT0s[:, N:2048], 0.0)
            Xs1 = pa.tile([L, N], F16)
            A_sb = [pa.tile([128, NGP], F16, name=f"A{i}")
                    for i in range(NB)]
            V_sb = pa.tile([128, NB * L], F16)
            HT = pa.tile([T, NH], F16)
            nc.vector.memset(HT[:, N:NH], 0.0)
            o_sb = pa.tile([128, 16], F32)
            gi_sb = pa.tile([L, 3 * T], F32)

            with tc.tile_pool(name="psA", bufs=1, space="PSUM") as psA:
                # spatial encoder
                for (c0, cw) in _chunks(N, 512):
                    ps1 = psA.tile([L, 512], F32, tag="ps1", name="ps1", bufs=2)
                    nc.tensor.matmul(ps1[:, 0:cw], spW1_sb[:],
                                     xsT_sb[:, c0:c0 + cw],
                                     start=True, stop=True)
                    elu_from_psum(nc, tmp, ps1[:, 0:cw], spb1_sb[:, 0:1],
                                  Xs1[:, c0:c0 + cw], "se")
                for (c0, cw) in _chunks(N, 512):
                    ps2 = psA.tile([L, 512], F32, tag="ps2", name="ps2", bufs=2)
                    nc.tensor.matmul(ps2[:, 0:cw], spW2_sb[:],
                                     Xs1[:, c0:c0 + cw],
                                     start=True, stop=True)
                    nc.scalar.activation(XT0s[:, c0:c0 + cw], ps2[:, 0:cw],
                                         AF.Identity, bias=spb2_sb[:, 0:1])

                # temporal encoder: gi for all steps (per-gate, base 0)
                for gg in range(3):
                    psgi = psA.tile([L, T], F32, tag="psgi",
                                    name=f"psgi{gg}")
                    for k in range(KC):
                        nc.tensor.matmul(
                            psgi[:],
                            tWihT_sb[:, k * 3 * L + L * gg:
                                     k * 3 * L + L * (gg + 1)],
                            xtT_sb[:, k * T:(k + 1) * T],
                            start=(k == 0), stop=(k == KC - 1))
                    nc.scalar.activation(gi_sb[:, T * gg:T * (gg + 1)],
                                         psgi[:], AF.Identity,
                                         bias=tgib_sb[:, gg:gg + 1])

                # temporal recurrence (hidden 32), fp16 state in XT0t cols
                h0_16 = tmp.tile([L, 1], F16, tag="h016", name="h016")
                nc.vector.memset(h0_16[:], 0.0)
                hcol = h0_16[:]
                for t in range(T):
                    psg = [psA.tile([L, 1], F32, tag=f"psg{gg}",
                                    bufs=1, name=f"psg{gg}_{t}")
                           for gg in range(3)]
                    for gg in range(3):
                        nc.tensor.matmul(psg[gg][:],
                                         tWhhT_sb[:, L * gg:L * (gg + 1)],
                                         hcol, start=True, stop=True)
                    r = tmp.tile([L, 1], F32, tag="t_r", name="t_r")
                    nc.scalar.activation(r[:], psg[0][:], AF.Sigmoid,
                                         bias=gi_sb[:, t:t + 1])
                    z = tmp.tile([L, 1], F32, tag="t_z", name="t_z")
                    nc.scalar.activation(z[:], psg[1][:], AF.Sigmoid,
                                         bias=gi_sb[:, T + t:T + t + 1])
                    t6 = tmp.tile([L, 1], F32, tag="t_t6", name="t_t6")
                    nc.vector.scalar_tensor_tensor(
                        t6[:], psg[2][:], tbhhn_sb[:, 0:1], r[:],
                        AluOpType.add, AluOpType.mult)
                    nt = tmp.tile([L, 1], F32, tag="t_nt", name="t_nt")
                    nc.scalar.activation(nt[:], t6[:], AF.Tanh,
                                         bias=gi_sb[:, 2 * T + t:2 * T + t + 1])
                    d = tmp.tile([L, 1], F32, tag="t_d", name="t_d")
                    nc.vector.tensor_tensor(d[:], nt[:], hcol,
                                            AluOpType.subtract)
                    zd = tmp.tile([L, 1], F32, tag="t_zd", name="t_zd")
                    nc.vector.tensor_tensor(zd[:], z[:], d[:],
                                            AluOpType.mult)
                    # h_new = nt - z*(nt - h) = (1-z)*nt + z*h
                    nc.vector.tensor_tensor(XT0t[:, t:t + 1], nt[:], zd[:],
                                            AluOpType.subtract)
                    hcol = XT0t[:, t:t + 1]

            # ---- TN layers ----
            # lhsT/rhs/output segments: seg 0 = temporal tile (128 cols),
            # segs 1+ = spatial tile (4x512). Spatial-only matmuls are
            # emitted first so they can overlap the temporal recurrence.
            XTat, XTas, XTbt, XTbs = XT0t, XT0s, XT1t, XT1s

            def _lhs(i, xtt, xts):
                return xtt[:, 0:128] if i == 0 else                     xts[:, 128 * (i - 1):128 * i]

            def _segs(xtt, xts):
                segs = [(xtt, 0, 0, 128)]
                for (c0, cw) in _chunks(2048, 512):
                    segs.append((xts, c0, 128 + c0, cw))
                return segs

            for ti in range(2):
                with tc.tile_pool(name=f"psS{ti}", bufs=8,
                                  space="PSUM") as psS:
                    segs = _segs(XTat, XTas)
                    order = [(i, s) for i in range(1, NB)
                             for s in range(1, 5)]
                    order += [(0, s) for s in range(5)]
                    order += [(i, 0) for i in range(1, NB)]
                    for (i, s) in order:
                        (xt, c0, a0, cw) = segs[s]
                        ps = psS.tile([128, 512], F32, tag="psS",
                                      name="psS")
                        nc.tensor.matmul(ps[:, 0:cw],
                                         _lhs(i, XTat, XTas),
                                         xt[:, c0:c0 + cw],
                                         start=True, stop=True)
                        th = tmp.tile([128, 512], F16, tag="th",
                                      name="th")
                        nc.scalar.activation(th[:, 0:cw], ps[:, 0:cw],
                                             AF.Tanh)
                        nc.vector.tensor_scalar_max(A_sb[i][:, a0:a0 + cw],
                                                    th[:, 0:cw], 0.0)
                for li in range(2):
                    w0 = 32 * (2 * ti + li)
                    with tc.tile_pool(name=f"psG{ti}{li}", bufs=4,
                                      space="PSUM") as psG:
                        for j in list(range(1, NB)) + [0]:
                            psv = psG.tile([128, L], F32, tag="psv",
                                           name="psv")
                            nc.tensor.matmul(psv[:],
                                             _lhs(j, XTat, XTas),
                                             gcnW_sb[:, w0:w0 + L],
                                             start=True, stop=True)
                            nc.vector.tensor_copy(
                                V_sb[:, L * j:L * (j + 1)], psv[:])
                        for (xt, c0, a0, cw) in _segs(XTbt, XTbs):
                            psa = psG.tile([L, 512], F32, tag="psa",
                                           name="psa")
                            for j in range(NB):
                                nc.tensor.matmul(psa[:, 0:cw],
                                                 V_sb[:, L * j:L * (j + 1)],
                                                 A_sb[j][:, a0:a0 + cw],
                                                 start=(j == 0),
                                                 stop=(j == NB - 1))
                            elu_from_psum(nc, tmp, psa[:, 0:cw],
                                          gcnB_sb[:, 2 * ti + li:2 * ti + li + 1],
                                          xt[:, c0:c0 + cw], "ge")
                    nc.vector.memset(XTbt[:, T:128], 0.0)
                    nc.vector.memset(XTbs[:, N:2048], 0.0)
                    XTat, XTbt = XTbt, XTat
                    XTas, XTbs = XTbs, XTas

            # ---- feature rebuild + o_s ----
            with tc.tile_pool(name="psF", bufs=4, space="PSUM") as psF:
                for (c0, cw) in _chunks(N, 512):
                    psh = psF.tile([T, 512], F32, tag="psh", name="psh")
                    nc.tensor.matmul(psh[:, 0:cw], frW1_sb[:],
                                     XTas[:, c0:c0 + cw],
                                     start=True, stop=True)
                    elu_from_psum(nc, tmp, psh[:, 0:cw], frb1_sb[:, 0:1],
                                  HT[:, c0:c0 + cw], "fr")
                for j in range(16):
                    pso = psF.tile([128, 1], F32, tag="pso", name="pso")
                    nc.tensor.matmul(pso[:], HT[:, 128 * j:128 * (j + 1)],
                                     fvv_sb[:], start=True, stop=True)
                    nc.vector.tensor_copy(o_sb[:, j:j + 1], pso[:])
            nc.sync.dma_start(
                out_d[:].rearrange("(j p) one -> p j one", p=128),
                o_sb[:].rearrange("p (j one) -> p j one", j=16))

            # ---- AllGather Xt2, then gi for the big GRU ----
            xt2_d = dpool.tile([L, T], F16)
            nc.sync.dma_start(xt2_d[:], XTat[:, 0:T])
            xt2a_d = dpool.tile([NC * L, T], F16)
            nc.gpsimd.collective_compute(
                "AllGather", AluOpType.bypass,
                replica_groups=[list(range(NC))],
                ins=[xt2_d[:].opt()], outs=[xt2a_d[:].opt()])
            xt2a_sb = pa.tile([L, NC * T], F16)
            nc.sync.dma_start(
                xt2a_sb[:].rearrange("f (b t) -> f b t", b=NC),
                xt2a_d[:].rearrange("(b f) t -> f b t", f=L))

            # gi for the big GRU: per gate g, (128, 16*T) tiles with
            # column layout [t][half h][batch b] -> col 16t + 8h + b
            gi_g = [pa.tile([128, 16 * T], F32, name=f"gig{g}")
                    for g in range(3)]
            with tc.tile_pool(name="psI", bufs=6, space="PSUM") as psI:
                for b in range(NC):
                    for g in range(3):
                        for h in range(2):
                            m = 2 * g + h
                            psi = psI.tile([128, T], F32, tag="psi",
                                           name="psi")
                            nc.tensor.matmul(
                                psi[:], wihTs_sb[:, 128 * m:128 * (m + 1)],
                                xt2a_sb[:, T * b:T * (b + 1)],
                                start=True, stop=True)
                            nc.scalar.activation(
                                gi_g[g][:, 8 * h + b:16 * T:16], psi[:],
                                AF.Identity, bias=gibs_sb[:, m:m + 1])

            # ---- phase B: hidden-sharded big GRU ----
            # h slice state: (128, 16) f16 = [half h][batch b] columns
            hprev = pb.tile([128, 16], F16, tag="h16", name="h16_0")
            nc.vector.memset(hprev[:], 0.0)
            accv = pb.tile([128, 16], F32, tag="acc", name="acc_0")
            nc.vector.memset(accv[:], 0.0)
            hall_lo = pb.tile([128, 64], F16, tag="hlo", name="hlo_i")
            hall_hi = pb.tile([128, 64], F16, tag="hhi", name="hhi_i")
            nc.vector.memset(hall_lo[:], 0.0)
            nc.vector.memset(hall_hi[:], 0.0)

            with tc.tile_pool(name="psB", bufs=6, space="PSUM") as psB:
                for t in range(T):
                    pst = [psB.tile([128, 16], F32, tag=f"psB{g}",
                                    name=f"ps_{t}_{g}", bufs=2)
                           for g in range(3)]
                    # all lo-half contraction chunks first (across every
                    # gate group), so the hi-half hall DMA overlaps them
                    for klo in (True, False):
                        for g in range(3):
                            for h in range(2):
                                m = 2 * g + h
                                for k in (range(8) if klo else range(8, KC)):
                                    j = m * KC + k
                                    rhs = (hall_lo[:, 8 * k:8 * k + 8]
                                           if k < 8 else
                                           hall_hi[:, 8 * (k - 8):
                                                    8 * (k - 7)])
                                    nc.tensor.matmul(
                                        pst[g][:, 8 * h:8 * h + 8],
                                        wt_sb[:, 128 * j:128 * (j + 1)],
                                        rhs,
                                        start=(k == 0),
                                        stop=(k == KC - 1))
                    gi_t = [gi_g[g][:, 16 * t:16 * t + 16] for g in range(3)]
                    rp = tmp.tile([128, 16], F32, tag="b_rp", name="b_rp")
                    nc.vector.tensor_tensor(rp[:], pst[0][:], gi_t[0],
                                            AluOpType.add)
                    r = tmp.tile([128, 16], F32, tag="b_r", name="b_r")
                    nc.scalar.activation(r[:], rp[:], AF.Sigmoid)
                    zp = tmp.tile([128, 16], F32, tag="b_zp", name="b_zp")
                    nc.vector.tensor_tensor(zp[:], pst[1][:], gi_t[1],
                                            AluOpType.add)
                    z = tmp.tile([128, 16], F32, tag="b_z", name="b_z")
                    nc.scalar.activation(z[:], zp[:], AF.Sigmoid)
                    bn = tmp.tile([128, 16], F32, tag="b_bn", name="b_bn")
                    nc.vector.tensor_tensor(bn[:], pst[2][:], bhhn16_sb[:],
                                            AluOpType.add)
                    t6 = tmp.tile([128, 16], F32, tag="b_t6", name="b_t6")
                    nc.vector.tensor_tensor(t6[:], bn[:], r[:],
                                            AluOpType.mult)
                    npre = tmp.tile([128, 16], F32, tag="b_np", name="b_np")
                    nc.vector.tensor_tensor(npre[:], t6[:], gi_t[2],
                                            AluOpType.add)
                    nt = tmp.tile([128, 16], F32, tag="b_nt", name="b_nt")
                    nc.scalar.activation(nt[:], npre[:], AF.Tanh)
                    d = tmp.tile([128, 16], F32, tag="b_d", name="b_d")
                    nc.vector.tensor_tensor(d[:], hprev[:], nt[:],
                                            AluOpType.subtract)
                    zd = tmp.tile([128, 16], F32, tag="b_zd", name="b_zd")
                    nc.vector.tensor_tensor(zd[:], z[:], d[:],
                                            AluOpType.mult)
                    hnew = pb.tile([128, 16], F16, tag="h16",
                                   name=f"hnew_{t}")
                    nc.vector.tensor_tensor(hnew[:], nt[:], zd[:],
                                            AluOpType.add)
                    if t < T - 1:
                        agin = dpool.tile([128, 16], F16, tag=f"agi{t}",
                                          name=f"agi{t}")
                        nc.sync.dma_start(agin[:], hnew[:])
                    anew = pb.tile([128, 16], F32, tag="acc",
                                   name=f"anew_{t}")
                    nc.vector.scalar_tensor_tensor(
                        anew[:], hnew[:], outWb_sb[:, t:t + 1], accv[:],
                        AluOpType.mult, AluOpType.add)
                    accv = anew
                    hprev = hnew
                    if t == T - 1:
                        break
                    agout = dpool.tile([NC * 128, 16], F16, tag=f"ago{t}",
                                       name=f"ago{t}")
                    nc.gpsimd.collective_compute(
                        "AllGather", AluOpType.bypass,
                        replica_groups=[list(range(NC))],
                        ins=[agin[:].opt()], outs=[agout[:].opt()],
                        unique_tensors="Yes")
                    hall_lo = pb.tile([128, 64], F16, tag="hlo",
                                      name=f"hlo{t}")
                    hall_hi = pb.tile([128, 64], F16, tag="hhi",
                                      name=f"hhi{t}")
                    nc.sync.dma_start(
                        hall_lo[:].rearrange("p (c m) -> p c m", c=4),
                        agout[0:512, :].rearrange("(c p) m -> p c m", p=128))
                    nc.sync.dma_start(
                        hall_hi[:].rearrange("p (c m) -> p c m", c=4),
                        agout[512:1024, :].rearrange("(c p) m -> p c m",
                                                     p=128))

            # ---- write acc slice; host does the final combine ----
            nc.sync.dma_start(acc_d[:], accv[:])

    nc.compile()
    return nc


def _prep_inputs(inputs):
    f16 = np.float16
    f32 = np.float32
    x = np.asarray(inputs['x'], f32)
    x_mark = np.asarray(inputs['x_mark'], f32)
    g = {k: np.asarray(v, f32) for k, v in inputs.items()}

    # temporal-GRU input weights, transposed + padded to NH rows
    tWihT = np.zeros((NH, 3 * L), f16)
    tWihT[:N + Dt + E, :] = g['t_Wih'].T.astype(f16)
    rz_mask = np.concatenate([np.ones(2 * L, f32), np.zeros(L, f32)])
    tgib = (g['t_bih'] + g['t_bhh'] * rz_mask).reshape(3, L).T.astype(f32)

    # big GRU: pad hidden 2000->2048, group gates [r;z;n] per slice
    Whh_p = np.zeros((3 * NH, NH), f32)
    Wih_p = np.zeros((3 * NH, L), f32)
    bih_p = np.zeros(3 * NH, f32)
    bhh_p = np.zeros(3 * NH, f32)
    for gi in range(3):
        Whh_p[gi * NH:gi * NH + N, :N] = g['tr_Whh'][gi * N:(gi + 1) * N, :]
        Wih_p[gi * NH:gi * NH + N, :] = g['tr_Wih'][gi * N:(gi + 1) * N, :]
        bih_p[gi * NH:gi * NH + N] = g['tr_bih'][gi * N:(gi + 1) * N]
        bhh_p[gi * NH:gi * NH + N] = g['tr_bhh'][gi * N:(gi + 1) * N]

    fvv = (g['fr_W2'] @ g['out_W']).astype(f16)
    fb = float((g['fr_b2'] @ g['out_W'])[0] + g['out_b'][0])
    outWb = np.tile(g['out_W'][:, 0][None, :], (128, 1)).astype(f32)

    common = {
        'spembT': g['sp_emb'].T.astype(f16),
        'spW1': g['sp_W1'].astype(f16),
        'spb1': g['sp_b1'][:, None].astype(f32),
        'spW2': g['sp_W2'].astype(f16),
        'spb2': g['sp_b2'][:, None].astype(f32),
        'tWihT': tWihT,
        'tWhhT': g['t_Whh'].T.astype(f16),
        'tgib': tgib,
        'tbhhn': g['t_bhh'][2 * L:3 * L][:, None].astype(f32),
        'gcnW': np.concatenate([g['gcn_W'][ti, li] for ti in range(2) for li in range(2)], axis=1).astype(f16),
        'gcnB': np.stack([g['gcn_b'][ti, li] for ti in range(2) for li in range(2)], axis=1).astype(f32),
        'frW1': g['fr_W1'].astype(f16),
        'frb1': g['fr_b1'][:, None].astype(f32),
        'fvv': fvv,
        'outWb': outWb,
    }

    in_maps = []
    for c in range(NC):
        rows = np.concatenate([np.arange(c * SL, (c + 1) * SL) + gi * NH
                               for gi in range(3)])
        xtT = np.zeros((NH, T), f16)
        xtT[:N, :] = x[c].T.astype(f16)
        xtT[N:N + Dt, :] = x_mark[c].T.astype(f16)
        xtT[N + Dt:N + Dt + E, :] = g['t_emb'].T.astype(f16)
        m = dict(common)
        m['xT'] = x[c].astype(f16)
        m['xtT'] = xtT
        m['wihTs'] = Wih_p[rows, :].T.astype(f16).copy()
        m['whhTs'] = Whh_p[rows, :].T.astype(f16).copy()
        gib = bih_p[rows] + bhh_p[rows] * np.concatenate(
            [np.ones(2 * SL, f32), np.zeros(SL, f32)])
        m['gibs'] = gib[:, None].astype(f32)
        # bhh_n for this slice, (128 rows, [half h][batch b]) columns
        bn = bhh_p[rows[2 * SL:]]  # (256,)
        m['bhhn16'] = np.concatenate(
            [np.tile(bn[128 * h:128 * (h + 1)][:, None], (1, 8))
             for h in range(2)], axis=1).astype(f32)
        in_maps.append(m)
    return in_maps, fb


def kernel(**inputs):
    if 'nc' not in _CACHE:
        _CACHE['nc'] = build_program()
    nc = _CACHE['nc']
    in_maps, fb = _prep_inputs(inputs)
    res = run_bass_kernel_spmd(nc, in_maps, core_ids=list(range(NC)))
    _CACHE['last_res'] = res
    o_s = np.stack([res.results[b]['out_d'][:N, 0] for b in range(NC)])
    # acc_d per core: (128, 16) cols = [half h][batch b]; hidden row
    # n = c*256 + h*128 + p
    acc_full = np.zeros((NH, B), np.float32)
    for c in range(NC):
        a = res.results[c]['acc_d']
        for h in range(2):
            acc_full[c * SL + 128 * h:c * SL + 128 * (h + 1), :] = \
                a[:, 8 * h:8 * h + 8]
    out = o_s + acc_full[:N, :].T + fb
    return out[:, :, None].astype(np.float32)
